# revision 20
# baseline (speedup 1.0000x reference)
"""Trainium2 Bass kernel for nn_DeepHeuristicHypergraphAttention.

Data-parallel over batch B=64 across 8 NeuronCores (8 batch rows per core).

Per (b,e):  node = LN(ent + role_emb[roles]) (affine folded into Wk)
  q = gelu(q_emb@Wq), k = gelu(node@Wk')             per head h (D=128)
  h1 = LN1(gelu([q,k,|q-k|,q*k] @ s1_w))             (LN1 affine folded into s2)
  h2pre = gelu(h1n @ s2_w')                          (LN2 folded into s3)
  base = r2*(h2pre.s3w' - m2*sum(s3w')) + c
  out[b] = sigmoid(mean_h sum_e gate*mask*base)

Device mapping: all heavy matmuls bf16 on PE; gelu/tanh/identity on ACT (one
table set); stats via bn_stats + pooled combine; rsqrt via Newton on ALU
engines; row<->transposed layout flips via DMA xbar transpose (bf16).
"""

import sys

for _p in ("/opt/trn_rl_repo",):
    if _p not in sys.path:
        sys.path.insert(0, _p)

from contextlib import ExitStack

import numpy as np
import ml_dtypes

import concourse.bass as bass
import concourse.tile as tile
from concourse import bacc, mybir
from concourse.bass_utils import run_bass_kernel_spmd

F32 = mybir.dt.float32
BF16 = mybir.dt.bfloat16
I32 = mybir.dt.int32
AF = mybir.ActivationFunctionType
ALU = mybir.AluOpType
AX = mybir.AxisListType

B, E, H, D, EMB = 64, 256, 8, 128, 768
NCORES = 8
BPC = B // NCORES  # 8 batch rows per core
EPS = 1e-5
MAGIC = 0x5F3759DF

BF = np.dtype(ml_dtypes.bfloat16)


def _bf(x):
    return np.ascontiguousarray(np.asarray(x, np.float32).astype(BF))


def _f32(x):
    return np.ascontiguousarray(np.asarray(x, np.float32))


def fold_params(p):
    """Host-side folding of LN affine params into adjacent matmul weights."""
    fn_g, fn_b = _f32(p["fn_g"]), _f32(p["fn_b"])
    Wk, bk = _f32(p["Wk"]), _f32(p["bk"])
    Wq, bq = _f32(p["Wq"]), _f32(p["bq"])
    s1_w, s1_b = _f32(p["s1_w"]), _f32(p["s1_b"])
    ln1_g, ln1_b = _f32(p["ln1_g"]), _f32(p["ln1_b"])
    s2_w, s2_b = _f32(p["s2_w"]), _f32(p["s2_b"])
    ln2_g, ln2_b = _f32(p["ln2_g"]), _f32(p["ln2_b"])
    s3_w, s3_b = _f32(p["s3_w"]), _f32(p["s3_b"])

    wk_f = fn_g[:, None] * Wk
    bk_f = bk + fn_b @ Wk
    s2_f = ln1_g[:, None] * s2_w
    s2b_f = s2_b + ln1_b @ s2_w
    s3_f = ln2_g * s3_w[:, 0]
    s3c = float(s3_b[0] + ln2_b @ s3_w[:, 0])

    return {
        "wk": _bf(wk_f),
        "wq": _bf(Wq),
        "bk": _f32(bk_f),
        "bq": _f32(bq),
        "wa": _bf(s1_w[0:D]),
        "wb": _bf(s1_w[D : 2 * D]),
        "wc": _bf(s1_w[2 * D : 3 * D]),
        "wd": _bf(s1_w[3 * D :]),
        "s1b": _f32(s1_b),
        "w2": _bf(s2_f),
        "s2b": _f32(s2b_f),
        "s3w4": _bf(np.tile(s3_f, 4)),
        "s3c": s3c,
        "role_emb": _f32(p["role_emb"]),
        "idf_w": float(np.asarray(p["idf_w"]).reshape(-1)[0]),
        "idf_b": float(np.asarray(p["idf_b"]).reshape(-1)[0]),
    }


def build_program(have):
    """Build the SPMD bass program; `have` flags enable bias paths."""
    nc = bacc.Bacc(
        "TRN2",
        target_bir_lowering=False,
        debug=False,
        enable_asserts=False,
        num_devices=NCORES,
    )

    ent_d = nc.dram_tensor("ent", [BPC, E, EMB], BF16, kind="ExternalInput")
    q8_d = nc.dram_tensor("q8", [BPC, EMB], BF16, kind="ExternalInput")
    roles_d = nc.dram_tensor("rolesf", [BPC, E], F32, kind="ExternalInput")
    idfs_d = nc.dram_tensor("idfs", [BPC, E], F32, kind="ExternalInput")
    mask_d = nc.dram_tensor("maskf", [BPC, E], F32, kind="ExternalInput")
    wk_d = nc.dram_tensor("wk", [EMB, H * D], BF16, kind="ExternalInput")
    wq_d = nc.dram_tensor("wq", [EMB, H * D], BF16, kind="ExternalInput")
    wa_d = nc.dram_tensor("wa", [D, 2 * D], BF16, kind="ExternalInput")
    wb_d = nc.dram_tensor("wb", [D, 2 * D], BF16, kind="ExternalInput")
    wc_d = nc.dram_tensor("wc", [D, 2 * D], BF16, kind="ExternalInput")
    wd_d = nc.dram_tensor("wd", [D, 2 * D], BF16, kind="ExternalInput")
    w2_d = nc.dram_tensor("w2", [2 * D, D], BF16, kind="ExternalInput")
    s3w4_d = nc.dram_tensor("s3w4", [1, 4 * D], BF16, kind="ExternalInput")
    role_d = nc.dram_tensor("role_emb", [6, EMB], F32, kind="ExternalInput")
    brows_d = nc.dram_tensor("brows", [4, H * D], BF16, kind="ExternalInput")
    sc_d = nc.dram_tensor("scal", [1, 8], F32, kind="ExternalInput")
    iota_d = nc.dram_tensor("iota6", [6, 128], F32, kind="ExternalInput")
    out_d = nc.dram_tensor("out", [BPC], F32, kind="ExternalOutput")

    with tile.TileContext(nc) as tc, ExitStack() as ctx:
        cpool = ctx.enter_context(tc.tile_pool(name="consts", bufs=1))
        spool = ctx.enter_context(tc.tile_pool(name="setup", bufs=1))
        sdma = nc.sync
        gp = nc.gpsimd
        dve = nc.vector

        # ---------- small helpers ----------
        def newton_rsqrt(pool, v, ncols, tag, iters=3):
            """y = 1/sqrt(v) on [128, ncols] fp32 (v > 0), no ACT tables."""
            ib = pool.tile([128, ncols], I32, tag=f"nt_i_{tag}")
            magic = pool.tile([128, ncols], I32, tag=f"nt_m_{tag}")
            dve.memset(magic[:], MAGIC)
            dve.tensor_scalar(
                ib[:], v[:].bitcast(I32), 1, None, ALU.arith_shift_right
            )
            dve.scalar_tensor_tensor(
                ib[:], magic[:], 1.0, ib[:], ALU.bypass, ALU.subtract
            )
            y = pool.tile([128, ncols], F32, tag=f"nt_y_{tag}")
            dve.tensor_copy(y[:], ib[:].bitcast(F32))
            t = pool.tile([128, ncols], F32, tag=f"nt_t_{tag}")
            for _ in range(iters):
                dve.tensor_tensor(t[:], y[:], y[:], ALU.mult)
                dve.tensor_tensor(t[:], t[:], v[:], ALU.mult)
                dve.tensor_scalar(t[:], t[:], -0.5, 1.5, ALU.mult, ALU.add)
                dve.tensor_tensor(y[:], y[:], t[:], ALU.mult)
            return y

        def pool_stats(pool, sview, n_pop, subtot, tag):
            """(m, var) from bn_stats 6-tuples.

            sview: [128, n_pop, 6] AP; each population = even half (count
            subtot/2, mean at [...,1], cnt*var at [...,2]) + odd half
            ([...,4], [...,5]).  m = (me+mo)/2 ;
            E[x^2] = (cve+cvo)/subtot + (me^2+mo^2)/2 ; var = E[x^2]-m^2.
            """
            me = sview[:, :, 1:2]
            mo = sview[:, :, 4:5]
            cve = sview[:, :, 2:3]
            cvo = sview[:, :, 5:6]
            m = pool.tile([128, n_pop], F32, tag=f"st_m_{tag}")
            ex2 = pool.tile([128, n_pop], F32, tag=f"st_e_{tag}")
            sq = pool.tile([128, n_pop], F32, tag=f"st_q_{tag}")
            v = pool.tile([128, n_pop], F32, tag=f"st_v_{tag}")
            m3 = m[:].rearrange("p (n o) -> p n o", o=1)
            dve.tensor_tensor(m3, me, mo, ALU.add)
            dve.tensor_scalar(m[:], m[:], 0.5, None, ALU.mult)
            e3 = ex2[:].rearrange("p (n o) -> p n o", o=1)
            q3 = sq[:].rearrange("p (n o) -> p n o", o=1)
            dve.tensor_tensor(e3, cve, cvo, ALU.add)
            dve.tensor_scalar(ex2[:], ex2[:], 1.0 / subtot, None, ALU.mult)
            dve.tensor_tensor(q3, me, me, ALU.mult)
            dve.scalar_tensor_tensor(ex2[:], sq[:], 0.5, ex2[:], ALU.mult, ALU.add)
            dve.tensor_tensor(q3, mo, mo, ALU.mult)
            dve.scalar_tensor_tensor(ex2[:], sq[:], 0.5, ex2[:], ALU.mult, ALU.add)
            dve.tensor_tensor(sq[:], m[:], m[:], ALU.mult)
            dve.tensor_tensor(v[:], ex2[:], sq[:], ALU.subtract)
            return m, v

        def pool_pairs(pool, m4, v4, n_out, tag):
            """Merge adjacent population pairs: [128, 2*n_out] -> [128, n_out]."""
            m = pool.tile([128, n_out], F32, tag=f"pp_m_{tag}")
            v = pool.tile([128, n_out], F32, tag=f"pp_v_{tag}")
            e4 = pool.tile([128, 2 * n_out], F32, tag=f"pp_e_{tag}")
            sq = pool.tile([128, n_out], F32, tag=f"pp_q_{tag}")
            dve.tensor_tensor(e4[:], m4[:], m4[:], ALU.mult)
            dve.tensor_tensor(e4[:], e4[:], v4[:], ALU.add)  # E[x^2] per sub-pop
            m2v = m4[:].rearrange("p (n t) -> p n t", t=2)
            dve.reduce_sum(m[:], m2v, axis=AX.X)
            dve.tensor_scalar(m[:], m[:], 0.5, None, ALU.mult)
            e2v = e4[:].rearrange("p (n t) -> p n t", t=2)
            dve.reduce_sum(v[:], e2v, axis=AX.X)
            dve.tensor_scalar(v[:], v[:], 0.5, None, ALU.mult)
            dve.tensor_tensor(sq[:], m[:], m[:], ALU.mult)
            dve.tensor_tensor(v[:], v[:], sq[:], ALU.subtract)
            return m, v

        # ---------- constants / params ----------
        wk_t, wq_t = [], []
        for c in range(6):
            t = cpool.tile([128, H * D], BF16, tag=f"wk{c}")
            sdma.dma_start(t[:], wk_d[c * 128 : (c + 1) * 128, :])
            wk_t.append(t)
            t = cpool.tile([128, H * D], BF16, tag=f"wq{c}")
            sdma.dma_start(t[:], wq_d[c * 128 : (c + 1) * 128, :])
            wq_t.append(t)
        wa_t = cpool.tile([128, 2 * D], BF16, tag="wa")
        sdma.dma_start(wa_t[:], wa_d[:, :])
        wbcd = []
        for nm, dd in (("wb", wb_d), ("wc", wc_d), ("wd", wd_d)):
            t = cpool.tile([128, 2 * D], BF16, tag=nm)
            sdma.dma_start(t[:], dd[:, :])
            wbcd.append(t)
        w2_t = []
        for c in range(2):
            t = cpool.tile([128, D], BF16, tag=f"w2{c}")
            sdma.dma_start(t[:], w2_d[c * 128 : (c + 1) * 128, :])
            w2_t.append(t)
        role_t = cpool.tile([6, EMB], F32, tag="role")
        sdma.dma_start(role_t[:], role_d[:, :])
        s3w4row = cpool.tile([1, 4 * D], BF16, tag="s3w4r")
        sdma.dma_start(s3w4row[:], s3w4_d[:, :])
        scrow = cpool.tile([1, 8], F32, tag="scrow")
        sdma.dma_start(scrow[:], sc_d[:, :])
        # bias rows (each on partition 0): 0=bk', 1=bq, 2=s1_b, 3=s2b4
        brow_t = []
        for r in range(4):
            t = cpool.tile([1, H * D], BF16, tag=f"brow{r}")
            sdma.dma_start(t[:], brows_d[r : r + 1, :])
            brow_t.append(t)

        ones1_128b = cpool.tile([1, 128], BF16, tag="o128b")
        dve.memset(ones1_128b[:], 1.0)
        ones1_256b = cpool.tile([1, 256], BF16, tag="o256b")
        dve.memset(ones1_256b[:], 1.0)
        ones1_6 = cpool.tile([1, 6], F32, tag="o6")
        dve.memset(ones1_6[:], 1.0)
        ones128_f = cpool.tile([128, 1], F32, tag="o128f")
        dve.memset(ones128_f[:], 1.0)
        ones1_128f = cpool.tile([1, 128], F32, tag="o128f1")
        dve.memset(ones1_128f[:], 1.0)
        iota6 = cpool.tile([6, 128], F32, tag="iota6")
        sdma.dma_start(iota6[:], iota_d[:, :])

        # ---------- one-time setup (own psum pool, released after) ----------
        with tc.tile_pool(name="psum_setup", bufs=1, space="PSUM") as pset:
            ps = pset.tile([128, 4 * D], F32, tag="bc")
            nc.tensor.matmul(ps[:], ones1_128b[:], s3w4row[:], start=True, stop=True)
            s3w_b4 = cpool.tile([128, 4 * D], BF16, tag="s3wb")
            dve.tensor_copy(s3w_b4[:], ps[:])
            ps2 = pset.tile([128, 8], F32, tag="bcs")
            nc.tensor.matmul(ps2[:], ones1_128f[:], scrow[:], start=True, stop=True)
            sc_b = cpool.tile([128, 8], F32, tag="scb")
            dve.tensor_copy(sc_b[:], ps2[:])
            w3bar = cpool.tile([128, 4], F32, tag="w3bar")
            dve.reduce_sum(
                w3bar[:], s3w_b4[:].rearrange("p (a b) -> p a b", a=4), axis=AX.X
            )

            # gate: gm = sigmoid(idf_w*log1p(idf)+idf_b) * mask, [128, b*2+eh]
            NB2 = 2 * BPC
            idf_t = spool.tile([128, NB2], F32, tag="idf")
            msk_t = spool.tile([128, NB2], F32, tag="msk")
            sdma.dma_start(
                idf_t[:], idfs_d[:, :].rearrange("b (j p) -> p (b j)", p=128)
            )
            sdma.dma_start(
                msk_t[:], mask_d[:, :].rearrange("b (j p) -> p (b j)", p=128)
            )
            tA = spool.tile([128, NB2], F32, tag="gA")
            tR = spool.tile([128, NB2], F32, tag="gR")
            tW = spool.tile([128, NB2], F32, tag="gW")
            tW2 = spool.tile([128, NB2], F32, tag="gW2")
            tH = spool.tile([128, NB2], F32, tag="gH")
            dve.tensor_scalar(tA[:], idf_t[:], 2.0, None, ALU.add)
            dve.reciprocal(tR[:], tA[:])
            dve.tensor_tensor(tW[:], idf_t[:], tR[:], ALU.mult)
            dve.tensor_tensor(tW2[:], tW[:], tW[:], ALU.mult)
            dve.memset(tH[:], 1.0 / 9.0)
            for cc in (1.0 / 7.0, 1.0 / 5.0, 1.0 / 3.0, 1.0):
                dve.tensor_tensor(tH[:], tH[:], tW2[:], ALU.mult)
                dve.tensor_scalar(tH[:], tH[:], cc, None, ALU.add)
            dve.tensor_tensor(tH[:], tH[:], tW[:], ALU.mult)
            dve.tensor_scalar(tH[:], tH[:], 2.0, None, ALU.mult)
            dve.tensor_scalar(
                tH[:], tH[:], sc_b[:, 0:1], sc_b[:, 1:2], ALU.mult, ALU.add
            )
            gate_t = spool.tile([128, NB2], F32, tag="gate")
            nc.scalar.activation(gate_t[:], tH[:], AF.Tanh, scale=0.5)
            dve.tensor_scalar(gate_t[:], gate_t[:], 0.5, 0.5, ALU.mult, ALU.add)
            gm_t = spool.tile([128, NB2], F32, tag="gm")
            dve.tensor_tensor(gm_t[:], gate_t[:], msk_t[:], ALU.mult)

            # q path (all b at once)
            qeT = spool.tile([128, 6, BPC], BF16, tag="qeT")
            with nc.allow_non_contiguous_dma("qeT transposed load"):
                for c in range(6):
                    sdma.dma_start(
                        qeT[:, c, :],
                        q8_d[:, c * 128 : (c + 1) * 128].rearrange("j p -> p j"),
                    )
            qgT = spool.tile([128, H * BPC], BF16, tag="qgT")
            qgF = spool.tile([128, H * BPC], F32, tag="qgF")
            for h in range(H):
                psq = pset.tile([128, BPC], F32, tag="ps_q")
                for c in range(6):
                    nc.tensor.matmul(
                        psq[:],
                        wq_t[c][:, h * 128 : (h + 1) * 128],
                        qeT[:, c, :],
                        start=(c == 0),
                        stop=(c == 5 and not have["bq"]),
                    )
                if have["bq"]:
                    nc.tensor.matmul(
                        psq[:],
                        brow_t[1][:, h * 128 : (h + 1) * 128],
                        ones1_128b[:, 0:BPC],
                        start=False,
                        stop=True,
                    )
                nc.scalar.activation(
                    qgT[:, h * BPC : (h + 1) * BPC], psq[:], AF.Gelu
                )
                dve.tensor_copy(
                    qgF[:, h * BPC : (h + 1) * BPC],
                    qgT[:, h * BPC : (h + 1) * BPC],
                )
            # qb_all[b, h, dup, :] = q_bh @ Wa (+ s1_b); duplicated x2 cols
            qb_all = spool.tile([BPC, H, 2, 2 * D], BF16, tag="qb_all")
            for h in range(H):
                psqb = pset.tile([BPC, 2 * D], F32, tag="ps_qb")
                nc.tensor.matmul(
                    psqb[:],
                    qgT[:, h * BPC : (h + 1) * BPC],
                    wa_t[:],
                    start=True,
                    stop=not have["s1b"],
                )
                if have["s1b"]:
                    nc.tensor.matmul(
                        psqb[:],
                        ones1_128b[:, 0:BPC],
                        brow_t[2][:, 0 : 2 * D],
                        start=False,
                        stop=True,
                    )
                dve.tensor_copy(qb_all[:, h, 0, :], psqb[:])
                dve.tensor_copy(qb_all[:, h, 1, :], psqb[:])

        gatedc = spool.tile([128, BPC * 16], F32, tag="gatedc")

        # ---------- main pipeline ----------
        with (
            tc.tile_pool(name="work", bufs=2) as wpool,
            tc.tile_pool(name="stats", bufs=2) as stp,
            tc.tile_pool(name="psum_main", bufs=2, space="PSUM") as pp,
        ):
            for b in range(BPC):
                # qterm rows for this b gathered onto partition 0: [1,(h,2*256)]
                qbrow = wpool.tile([1, H, 2 * 2 * D], BF16, tag="qbrow")
                sdma.dma_start(
                    qbrow[:],
                    qb_all[b : b + 1, :, :, :].rearrange("o h a n -> o h (a n)"),
                )
                # ---- stage 1: node = LN(ent + role_emb[roles]) ----
                rolesf = wpool.tile([1, E], F32, tag="rolesf")
                sdma.dma_start(rolesf[:], roles_d[b, :][None, :])
                statsX = stp.tile([128, 4, 6], F32, tag="statsX")
                xs = []
                for t in range(2):
                    ent_t = wpool.tile([128, EMB], BF16, tag="ent")
                    sdma.dma_start(ent_t[:], ent_d[b, t * 128 : (t + 1) * 128, :])
                    ps6 = pp.tile([6, 128], F32, tag="pt")
                    nc.tensor.matmul(
                        ps6[:],
                        ones1_6[:],
                        rolesf[:, t * 128 : (t + 1) * 128],
                        start=True,
                        stop=True,
                    )
                    oh = wpool.tile([6, 128], F32, tag="oh")
                    dve.tensor_tensor(oh[:], ps6[:], iota6[:], ALU.is_equal)
                    psx = pp.tile([128, EMB], F32, tag="px")
                    nc.tensor.matmul(
                        psx[:, 0:512], oh[:], role_t[:, 0:512], start=True, stop=True
                    )
                    nc.tensor.matmul(
                        psx[:, 512:768],
                        oh[:],
                        role_t[:, 512:768],
                        start=True,
                        stop=True,
                    )
                    x_t = wpool.tile([128, EMB], BF16, tag=f"x{t}")
                    dve.scalar_tensor_tensor(
                        x_t[:], ent_t[:], 1.0, psx[:], ALU.bypass, ALU.add
                    )
                    for g in range(2):
                        dve.bn_stats(
                            statsX[:, 2 * t + g, :],
                            x_t[:, g * 384 : (g + 1) * 384],
                        )
                    xs.append(x_t)
                m4, v4 = pool_stats(stp, statsX[:], 4, 384, "lnx")
                mX, vX = pool_pairs(stp, m4, v4, 2, "lnx")
                dve.tensor_scalar(vX[:], vX[:], EPS, None, ALU.add)
                rX = newton_rsqrt(stp, vX, 2, "lnx")
                nmrX = stp.tile([128, 2], F32, tag="nmrX")
                dve.scalar_tensor_tensor(
                    nmrX[:], mX[:], -1.0, rX[:], ALU.mult, ALU.mult
                )
                nodeT = wpool.tile([128, 2, 6, 128], BF16, tag="nodeT")
                for t in range(2):
                    yn = wpool.tile([128, EMB], BF16, tag="yn")
                    nc.scalar.activation(
                        yn[:],
                        xs[t][:],
                        AF.Identity,
                        bias=nmrX[:, t : t + 1],
                        scale=rX[:, t : t + 1],
                    )
                    sdma.dma_start_transpose(nodeT[:, t, :, :], yn[:])

                # ---- stage 2: kT[h] = gelu(wk'.T @ nodeT) ----
                kT = []
                for hp in range(4):
                    psk = pp.tile([128, 512], F32, tag="pm")
                    for hh in range(2):
                        h = 2 * hp + hh
                        for c in range(6):
                            nc.tensor.matmul(
                                psk[:, hh * 256 : (hh + 1) * 256],
                                wk_t[c][:, h * 128 : (h + 1) * 128],
                                nodeT[:, :, c, :],
                                start=(c == 0),
                                stop=(c == 5 and not have["bk"]),
                            )
                        if have["bk"]:
                            nc.tensor.matmul(
                                psk[:, hh * 256 : (hh + 1) * 256],
                                brow_t[0][:, h * 128 : (h + 1) * 128],
                                ones1_256b[:],
                                start=False,
                                stop=True,
                            )
                    kt = wpool.tile([128, 512], BF16, tag=f"kT{hp}")
                    nc.scalar.activation(kt[:], psk[:], AF.Gelu)
                    kT.append(kt)

                # ---- stage 3: interaction chunks (transposed) ----
                int1 = wpool.tile([128, H * E], BF16, tag="int1")
                int2 = wpool.tile([128, H * E], BF16, tag="int2")
                for h in range(H):
                    ksl = kT[h // 2][:, (h % 2) * 256 : (h % 2) * 256 + 256]
                    qcol = qgF[:, h * BPC + b : h * BPC + b + 1]
                    dve.tensor_scalar(
                        int1[:, h * 256 : (h + 1) * 256], ksl, qcol, None,
                        ALU.subtract,
                    )
                    iv = int1[:, h * 256 : (h + 1) * 256].bitcast(mybir.dt.uint16)
                    dve.tensor_scalar(iv, iv, 0x7FFF, None, ALU.bitwise_and)
                    dve.tensor_scalar(
                        int2[:, h * 256 : (h + 1) * 256],
                        ksl,
                        qcol,
                        None,
                        ALU.mult,
                    )

                # ---- stage 4: s1 row-major + gelu + stats ----
                statsY = stp.tile([128, 16, 6], F32, tag="statsY")
                ybuf = []
                for sp in range(8):
                    h = sp
                    ps1 = pp.tile([128, 512], F32, tag="pm")
                    for half in range(2):
                        s_ = 2 * sp + half
                        base_col = (h % 2) * 256 + half * 128
                        lhs = [
                            kT[h // 2][:, base_col : base_col + 128],
                            int1[:, s_ * 128 : (s_ + 1) * 128],
                            int2[:, s_ * 128 : (s_ + 1) * 128],
                        ]
                        for kc in range(3):
                            nc.tensor.matmul(
                                ps1[:, half * 256 : (half + 1) * 256],
                                lhs[kc],
                                wbcd[kc][:],
                                start=(kc == 0),
                                stop=False,
                            )
                        nc.tensor.matmul(
                            ps1[:, half * 256 : (half + 1) * 256],
                            ones1_128b[:, 0:128],
                            qbrow[0:1, h, half * 256 : half * 256 + 256],
                            start=False,
                            stop=True,
                        )
                    yt = wpool.tile([128, 512], BF16, tag=f"y{sp}")
                    nc.scalar.activation(yt[:], ps1[:], AF.Gelu)
                    for g in range(2):
                        dve.bn_stats(
                            statsY[:, 2 * sp + g, :],
                            yt[:, g * 256 : (g + 1) * 256],
                        )
                    ybuf.append(yt)
                mY, vY = pool_stats(stp, statsY[:], 16, 256, "ln1")
                dve.tensor_scalar(vY[:], vY[:], EPS, None, ALU.add)
                rY = newton_rsqrt(stp, vY, 16, "ln1")
                nmrY = stp.tile([128, 16], F32, tag="nmrY")
                dve.scalar_tensor_tensor(
                    nmrY[:], mY[:], -1.0, rY[:], ALU.mult, ALU.mult
                )

                # ---- stage 5: normalize, transpose, s2, gelu, stats ----
                ynT = wpool.tile([128, 16, 2, 128], BF16, tag="ynT")
                for s_ in range(16):
                    yns = wpool.tile([128, 256], BF16, tag="yns")
                    dve.tensor_scalar(
                        yns[:],
                        ybuf[s_ // 2][:, (s_ % 2) * 256 : (s_ % 2) * 256 + 256],
                        rY[:, s_ : s_ + 1],
                        nmrY[:, s_ : s_ + 1],
                        ALU.mult,
                        ALU.add,
                    )
                    sdma.dma_start_transpose(ynT[:, s_, :, :], yns[:])
                statsZ = stp.tile([128, 16, 6], F32, tag="statsZ")
                zbuf = []
                for zt in range(4):
                    ps2 = pp.tile([128, 512], F32, tag="pm")
                    for j in range(4):
                        s_ = zt * 4 + j
                        for kc in range(2):
                            nc.tensor.matmul(
                                ps2[:, j * 128 : (j + 1) * 128],
                                ynT[:, s_, kc, :],
                                w2_t[kc][:],
                                start=(kc == 0),
                                stop=(kc == 1 and not have["s2b"]),
                            )
                        if have["s2b"]:
                            nc.tensor.matmul(
                                ps2[:, j * 128 : (j + 1) * 128],
                                ones1_128b[:, 0:128],
                                brow_t[3][:, 0:128],
                                start=False,
                                stop=True,
                            )
                    ztile = wpool.tile([128, 512], BF16, tag=f"z{zt}")
                    nc.scalar.activation(ztile[:], ps2[:], AF.Gelu)
                    for g in range(4):
                        dve.bn_stats(
                            statsZ[:, zt * 4 + g, :],
                            ztile[:, g * 128 : (g + 1) * 128],
                        )
                    zbuf.append(ztile)
                mZ, vZ = pool_stats(stp, statsZ[:], 16, D, "ln2")
                dve.tensor_scalar(vZ[:], vZ[:], EPS, None, ALU.add)
                rZ = newton_rsqrt(stp, vZ, 16, "ln2")

                # ---- stage 6: s3 dot + base + gate ----
                tcol = stp.tile([128, 16], F32, tag="tcol")
                for zt in range(4):
                    wsc = wpool.tile([128, 512], BF16, tag="wsc")
                    dve.tensor_tensor(wsc[:], zbuf[zt][:], s3w_b4[:], ALU.mult)
                    dve.reduce_sum(
                        tcol[:, zt * 4 : (zt + 1) * 4],
                        wsc[:].rearrange("p (g n) -> p g n", g=4),
                        axis=AX.X,
                    )
                base = stp.tile([128, 16], F32, tag="base")
                u = stp.tile([128, 16], F32, tag="ubase")
                dve.tensor_scalar(u[:], mZ[:], w3bar[:, 0:1], None, ALU.mult)
                dve.tensor_tensor(base[:], tcol[:], u[:], ALU.subtract)
                dve.tensor_tensor(base[:], base[:], rZ[:], ALU.mult)
                if have["s3c"]:
                    dve.tensor_scalar(base[:], base[:], sc_b[:, 2:3], None, ALU.add)
                bv = base[:].rearrange("p (h e) -> p h e", e=2)
                gv = gatedc[:, b * 16 : (b + 1) * 16].rearrange(
                    "p (h e) -> p h e", e=2
                )
                for eh in range(2):
                    dve.tensor_scalar(
                        gv[:, :, eh],
                        bv[:, :, eh],
                        gm_t[:, b * 2 + eh : b * 2 + eh + 1],
                        None,
                        ALU.mult,
                    )

            # ---- final reduction ----
            psf = pp.tile([1, BPC * 16], F32, tag="pt")
            nc.tensor.matmul(psf[:], ones128_f[:], gatedc[:], start=True, stop=True)
            hrow = spool.tile([1, BPC * 16], F32, tag="hrow")
            dve.tensor_copy(hrow[:], psf[:])
            srow = spool.tile([1, BPC], F32, tag="srow")
            dve.reduce_sum(
                srow[:], hrow[:].rearrange("p (b g) -> p b g", g=16), axis=AX.X
            )
            orow = spool.tile([1, BPC], F32, tag="orow")
            nc.scalar.activation(orow[:], srow[:], AF.Tanh, scale=1.0 / 16.0)
            dve.tensor_scalar(orow[:], orow[:], 0.5, 0.5, ALU.mult, ALU.add)
            sdma.dma_start(out_d[:], orow[:])

    nc.compile()
    return nc


def make_in_maps(inputs, fp):
    ent = _f32(inputs["ent_embs"]).reshape(NCORES, BPC, E, EMB)
    q8 = _f32(inputs["q_emb"]).reshape(NCORES, BPC, EMB)
    roles = np.asarray(inputs["roles"]).reshape(NCORES, BPC, E)
    idfs = _f32(inputs["idfs"]).reshape(NCORES, BPC, E)
    mask = np.asarray(inputs["mask"]).reshape(NCORES, BPC, E)
    scal = np.zeros(8, np.float32)
    scal[0] = fp["idf_w"]
    scal[1] = fp["idf_b"]
    scal[2] = fp["s3c"]
    brows = np.zeros((4, H * D), np.float32)
    brows[0] = fp["bk"]
    brows[1] = fp["bq"]
    brows[2, 0 : 2 * D] = fp["s1b"]
    brows[3, 0 : 4 * D] = np.tile(fp["s2b"], 4)

    shared = {
        "wk": fp["wk"],
        "wq": fp["wq"],
        "wa": fp["wa"],
        "wb": fp["wb"],
        "wc": fp["wc"],
        "wd": fp["wd"],
        "w2": fp["w2"],
        "s3w4": fp["s3w4"].reshape(1, -1),
        "role_emb": fp["role_emb"],
        "brows": brows.astype(BF),
        "scal": scal.reshape(1, 8),
        "iota6": np.repeat(np.arange(6, dtype=np.float32), 128).reshape(6, 128),
    }
    maps = []
    for c in range(NCORES):
        m = dict(shared)
        m["ent"] = ent[c].astype(BF)
        m["q8"] = q8[c].astype(BF)
        m["rolesf"] = roles[c].astype(np.float32)
        m["idfs"] = idfs[c]
        m["maskf"] = mask[c].astype(np.float32)
        maps.append(m)
    return maps


def _have_flags(fp):
    return {
        "bk": bool(np.any(fp["bk"])),
        "bq": bool(np.any(fp["bq"])),
        "s1b": bool(np.any(fp["s1b"])),
        "s2b": bool(np.any(fp["s2b"])),
        "s3c": bool(fp["s3c"] != 0.0),
    }


_CACHE = {}


def kernel(**inputs):
    fp = fold_params(inputs)
    have = _have_flags(fp)
    key = tuple(sorted(have.items()))
    if key not in _CACHE:
        _CACHE[key] = build_program(have)
    nc = _CACHE[key]
    in_maps = make_in_maps(inputs, fp)
    res = run_bass_kernel_spmd(nc, in_maps, list(range(NCORES)))
    out = np.concatenate([res.results[c]["out"] for c in range(NCORES)])
    return out.astype(np.float32)


# revision 22
# speedup vs baseline: 1.2458x; 1.2458x over previous
"""Trainium2 Bass kernel for nn_DeepHeuristicHypergraphAttention.

Data-parallel over batch B=64 across 8 NeuronCores (8 batch rows per core).

Per (b,e):  node = LN(ent + role_emb[roles]) (affine folded into Wk)
  q = gelu(q_emb@Wq), k = gelu(node@Wk')             per head h (D=128)
  h1 = LN1(gelu([q,k,|q-k|,q*k] @ s1_w))             (LN1 affine folded into s2)
  h2pre = gelu(h1n @ s2_w')                          (LN2 folded into s3)
  base = r2*(h2pre.s3w' - m2*sum(s3w')) + c
  out[b] = sigmoid(mean_h sum_e gate*mask*base)

Device mapping: all heavy matmuls bf16 on PE; gelu/tanh/identity on ACT (one
table set); stats via bn_stats + pooled combine; rsqrt via Newton on ALU
engines; row<->transposed layout flips via DMA xbar transpose (bf16).
"""

import sys

for _p in ("/opt/trn_rl_repo",):
    if _p not in sys.path:
        sys.path.insert(0, _p)

from contextlib import ExitStack

import numpy as np
import ml_dtypes

import concourse.bass as bass
import concourse.tile as tile
from concourse import bacc, mybir
from concourse.bass_utils import run_bass_kernel_spmd

F32 = mybir.dt.float32
BF16 = mybir.dt.bfloat16
I32 = mybir.dt.int32
AF = mybir.ActivationFunctionType
ALU = mybir.AluOpType
AX = mybir.AxisListType

B, E, H, D, EMB = 64, 256, 8, 128, 768
NCORES = 8
BPC = B // NCORES  # 8 batch rows per core
EPS = 1e-5
MAGIC = 0x5F3759DF

BF = np.dtype(ml_dtypes.bfloat16)


def _bf(x):
    return np.ascontiguousarray(np.asarray(x, np.float32).astype(BF))


def _f32(x):
    return np.ascontiguousarray(np.asarray(x, np.float32))


def fold_params(p):
    """Host-side folding of LN affine params into adjacent matmul weights."""
    fn_g, fn_b = _f32(p["fn_g"]), _f32(p["fn_b"])
    Wk, bk = _f32(p["Wk"]), _f32(p["bk"])
    Wq, bq = _f32(p["Wq"]), _f32(p["bq"])
    s1_w, s1_b = _f32(p["s1_w"]), _f32(p["s1_b"])
    ln1_g, ln1_b = _f32(p["ln1_g"]), _f32(p["ln1_b"])
    s2_w, s2_b = _f32(p["s2_w"]), _f32(p["s2_b"])
    ln2_g, ln2_b = _f32(p["ln2_g"]), _f32(p["ln2_b"])
    s3_w, s3_b = _f32(p["s3_w"]), _f32(p["s3_b"])

    wk_f = fn_g[:, None] * Wk
    bk_f = bk + fn_b @ Wk
    s2_f = ln1_g[:, None] * s2_w
    s2b_f = s2_b + ln1_b @ s2_w
    s3_f = ln2_g * s3_w[:, 0]
    s3c = float(s3_b[0] + ln2_b @ s3_w[:, 0])

    return {
        "wk": _bf(wk_f),
        "wq": _bf(Wq),
        "bk": _f32(bk_f),
        "bq": _f32(bq),
        "wa": _bf(s1_w[0:D]),
        "wb": _bf(s1_w[D : 2 * D]),
        "wc": _bf(s1_w[2 * D : 3 * D]),
        "wd": _bf(s1_w[3 * D :]),
        "s1b": _f32(s1_b),
        "w2": _bf(s2_f),
        "s2b": _f32(s2b_f),
        "s3w4": _bf(np.tile(s3_f, 4)),
        "s3c": s3c,
        "role_emb": _f32(p["role_emb"]),
        "idf_w": float(np.asarray(p["idf_w"]).reshape(-1)[0]),
        "idf_b": float(np.asarray(p["idf_b"]).reshape(-1)[0]),
    }


def build_program(have):
    """Build the SPMD bass program; `have` flags enable bias paths."""
    nc = bacc.Bacc(
        "TRN2",
        target_bir_lowering=False,
        debug=False,
        enable_asserts=False,
        num_devices=NCORES,
    )

    ent_d = nc.dram_tensor("ent", [BPC, E, EMB], BF16, kind="ExternalInput")
    q8_d = nc.dram_tensor("q8", [BPC, EMB], BF16, kind="ExternalInput")
    roles_d = nc.dram_tensor("rolesf", [BPC, E], BF16, kind="ExternalInput")
    idfs_d = nc.dram_tensor("idfs", [BPC, E], F32, kind="ExternalInput")
    mask_d = nc.dram_tensor("maskf", [BPC, E], F32, kind="ExternalInput")
    wk_d = nc.dram_tensor("wk", [EMB, H * D], BF16, kind="ExternalInput")
    wq_d = nc.dram_tensor("wq", [EMB, H * D], BF16, kind="ExternalInput")
    wa_d = nc.dram_tensor("wa", [D, 2 * D], BF16, kind="ExternalInput")
    wb_d = nc.dram_tensor("wb", [D, 2 * D], BF16, kind="ExternalInput")
    wc_d = nc.dram_tensor("wc", [D, 2 * D], BF16, kind="ExternalInput")
    wd_d = nc.dram_tensor("wd", [D, 2 * D], BF16, kind="ExternalInput")
    w2_d = nc.dram_tensor("w2", [2 * D, D], BF16, kind="ExternalInput")
    s3w4_d = nc.dram_tensor("s3w4", [1, 4 * D], BF16, kind="ExternalInput")
    role_d = nc.dram_tensor("role_emb", [6, EMB], BF16, kind="ExternalInput")
    brows_d = nc.dram_tensor("brows", [4, H * D], BF16, kind="ExternalInput")
    sc_d = nc.dram_tensor("scal", [1, 8], F32, kind="ExternalInput")
    iota_d = nc.dram_tensor("iota6", [6, 128], BF16, kind="ExternalInput")
    ident_d = nc.dram_tensor("ident", [128, 128], BF16, kind="ExternalInput")
    out_d = nc.dram_tensor("out", [BPC], F32, kind="ExternalOutput")

    with tile.TileContext(nc) as tc, ExitStack() as ctx:
        cpool = ctx.enter_context(tc.tile_pool(name="consts", bufs=1))
        spool = ctx.enter_context(tc.tile_pool(name="setup", bufs=1))
        sdma = nc.sync
        gp = nc.gpsimd
        dve = nc.vector

        # ---------- small helpers ----------
        def newton_rsqrt(pool, v, ncols, tag, iters=3):
            """y = 1/sqrt(v) on [128, ncols] fp32 (v > 0), no ACT tables."""
            ib = pool.tile([128, ncols], I32, tag=f"nt_i_{tag}")
            magic = pool.tile([128, ncols], I32, tag=f"nt_m_{tag}")
            dve.memset(magic[:], MAGIC)
            dve.tensor_scalar(
                ib[:], v[:].bitcast(I32), 1, None, ALU.arith_shift_right
            )
            dve.scalar_tensor_tensor(
                ib[:], magic[:], 1.0, ib[:], ALU.bypass, ALU.subtract
            )
            y = pool.tile([128, ncols], F32, tag=f"nt_y_{tag}")
            dve.tensor_copy(y[:], ib[:].bitcast(F32))
            t = pool.tile([128, ncols], F32, tag=f"nt_t_{tag}")
            for _ in range(iters):
                dve.tensor_tensor(t[:], y[:], y[:], ALU.mult)
                dve.tensor_tensor(t[:], t[:], v[:], ALU.mult)
                dve.tensor_scalar(t[:], t[:], -0.5, 1.5, ALU.mult, ALU.add)
                dve.tensor_tensor(y[:], y[:], t[:], ALU.mult)
            return y

        def pool_stats(pool, sview, n_pop, subtot, tag):
            """(m, var) from bn_stats 6-tuples.

            sview: [128, n_pop, 6] AP; each population = even half (count
            subtot/2, mean at [...,1], cnt*var at [...,2]) + odd half
            ([...,4], [...,5]).  m = (me+mo)/2 ;
            E[x^2] = (cve+cvo)/subtot + (me^2+mo^2)/2 ; var = E[x^2]-m^2.
            """
            me = sview[:, :, 1:2]
            mo = sview[:, :, 4:5]
            cve = sview[:, :, 2:3]
            cvo = sview[:, :, 5:6]
            m = pool.tile([128, n_pop], F32, tag=f"st_m_{tag}")
            ex2 = pool.tile([128, n_pop], F32, tag=f"st_e_{tag}")
            sq = pool.tile([128, n_pop], F32, tag=f"st_q_{tag}")
            v = pool.tile([128, n_pop], F32, tag=f"st_v_{tag}")
            m3 = m[:].rearrange("p (n o) -> p n o", o=1)
            dve.tensor_tensor(m3, me, mo, ALU.add)
            dve.tensor_scalar(m[:], m[:], 0.5, None, ALU.mult)
            e3 = ex2[:].rearrange("p (n o) -> p n o", o=1)
            q3 = sq[:].rearrange("p (n o) -> p n o", o=1)
            dve.tensor_tensor(e3, cve, cvo, ALU.add)
            dve.tensor_scalar(ex2[:], ex2[:], 1.0 / subtot, None, ALU.mult)
            dve.tensor_tensor(q3, me, me, ALU.mult)
            dve.scalar_tensor_tensor(ex2[:], sq[:], 0.5, ex2[:], ALU.mult, ALU.add)
            dve.tensor_tensor(q3, mo, mo, ALU.mult)
            dve.scalar_tensor_tensor(ex2[:], sq[:], 0.5, ex2[:], ALU.mult, ALU.add)
            dve.tensor_tensor(sq[:], m[:], m[:], ALU.mult)
            dve.tensor_tensor(v[:], ex2[:], sq[:], ALU.subtract)
            return m, v

        def pool_pairs(pool, m4, v4, n_out, tag):
            """Merge adjacent population pairs: [128, 2*n_out] -> [128, n_out]."""
            m = pool.tile([128, n_out], F32, tag=f"pp_m_{tag}")
            v = pool.tile([128, n_out], F32, tag=f"pp_v_{tag}")
            e4 = pool.tile([128, 2 * n_out], F32, tag=f"pp_e_{tag}")
            sq = pool.tile([128, n_out], F32, tag=f"pp_q_{tag}")
            dve.tensor_tensor(e4[:], m4[:], m4[:], ALU.mult)
            dve.tensor_tensor(e4[:], e4[:], v4[:], ALU.add)  # E[x^2] per sub-pop
            m2v = m4[:].rearrange("p (n t) -> p n t", t=2)
            dve.reduce_sum(m[:], m2v, axis=AX.X)
            dve.tensor_scalar(m[:], m[:], 0.5, None, ALU.mult)
            e2v = e4[:].rearrange("p (n t) -> p n t", t=2)
            dve.reduce_sum(v[:], e2v, axis=AX.X)
            dve.tensor_scalar(v[:], v[:], 0.5, None, ALU.mult)
            dve.tensor_tensor(sq[:], m[:], m[:], ALU.mult)
            dve.tensor_tensor(v[:], v[:], sq[:], ALU.subtract)
            return m, v

        # ---------- constants / params ----------
        wk_t, wq_t = [], []
        for c in range(6):
            t = cpool.tile([128, H * D], BF16, tag=f"wk{c}")
            sdma.dma_start(t[:], wk_d[c * 128 : (c + 1) * 128, :])
            wk_t.append(t)
            t = cpool.tile([128, H * D], BF16, tag=f"wq{c}")
            sdma.dma_start(t[:], wq_d[c * 128 : (c + 1) * 128, :])
            wq_t.append(t)
        wa_t = cpool.tile([128, 2 * D], BF16, tag="wa")
        sdma.dma_start(wa_t[:], wa_d[:, :])
        wbcd = []
        for nm, dd in (("wb", wb_d), ("wc", wc_d), ("wd", wd_d)):
            t = cpool.tile([128, 2 * D], BF16, tag=nm)
            sdma.dma_start(t[:], dd[:, :])
            wbcd.append(t)
        w2_t = []
        for c in range(2):
            t = cpool.tile([128, D], BF16, tag=f"w2{c}")
            sdma.dma_start(t[:], w2_d[c * 128 : (c + 1) * 128, :])
            w2_t.append(t)
        role_t = cpool.tile([6, EMB], BF16, tag="role")
        sdma.dma_start(role_t[:], role_d[:, :])
        s3w4row = cpool.tile([1, 4 * D], BF16, tag="s3w4r")
        sdma.dma_start(s3w4row[:], s3w4_d[:, :])
        scrow = cpool.tile([1, 8], F32, tag="scrow")
        sdma.dma_start(scrow[:], sc_d[:, :])
        # bias rows (each on partition 0): 0=bk', 1=bq, 2=s1_b, 3=s2b4
        brow_t = []
        for r in range(4):
            t = cpool.tile([1, H * D], BF16, tag=f"brow{r}")
            sdma.dma_start(t[:], brows_d[r : r + 1, :])
            brow_t.append(t)

        ones1_128b = cpool.tile([1, 128], BF16, tag="o128b")
        dve.memset(ones1_128b[:], 1.0)
        ones1_256b = cpool.tile([1, 256], BF16, tag="o256b")
        dve.memset(ones1_256b[:], 1.0)
        ones1_6 = cpool.tile([1, 6], BF16, tag="o6")
        dve.memset(ones1_6[:], 1.0)
        ones128_f = cpool.tile([128, 1], F32, tag="o128f")
        dve.memset(ones128_f[:], 1.0)
        ones1_128f = cpool.tile([1, 128], F32, tag="o128f1")
        dve.memset(ones1_128f[:], 1.0)
        iota6 = cpool.tile([6, 128], BF16, tag="iota6")
        sdma.dma_start(iota6[:], iota_d[:, :])
        ident = cpool.tile([128, 128], BF16, tag="ident")
        sdma.dma_start(ident[:], ident_d[:, :])

        # ---------- one-time setup (own psum pool, released after) ----------
        with tc.tile_pool(name="psum_setup", bufs=1, space="PSUM") as pset:
            ps = pset.tile([128, 4 * D], F32, tag="bc")
            nc.tensor.matmul(ps[:], ones1_128b[:], s3w4row[:], start=True, stop=True)
            s3w_b4 = cpool.tile([128, 4 * D], BF16, tag="s3wb")
            dve.tensor_copy(s3w_b4[:], ps[:])
            ps2 = pset.tile([128, 8], F32, tag="bcs")
            nc.tensor.matmul(ps2[:], ones1_128f[:], scrow[:], start=True, stop=True)
            sc_b = cpool.tile([128, 8], F32, tag="scb")
            dve.tensor_copy(sc_b[:], ps2[:])
            w3bar = cpool.tile([128, 4], F32, tag="w3bar")
            dve.reduce_sum(
                w3bar[:], s3w_b4[:].rearrange("p (a b) -> p a b", a=4), axis=AX.X
            )

            # gate: gm = sigmoid(idf_w*log1p(idf)+idf_b) * mask, [128, b*2+eh]
            NB2 = 2 * BPC
            idf_t = spool.tile([128, NB2], F32, tag="idf")
            msk_t = spool.tile([128, NB2], F32, tag="msk")
            sdma.dma_start(
                idf_t[:], idfs_d[:, :].rearrange("b (j p) -> p (b j)", p=128)
            )
            sdma.dma_start(
                msk_t[:], mask_d[:, :].rearrange("b (j p) -> p (b j)", p=128)
            )
            tA = spool.tile([128, NB2], F32, tag="gA")
            tR = spool.tile([128, NB2], F32, tag="gR")
            tW = spool.tile([128, NB2], F32, tag="gW")
            tW2 = spool.tile([128, NB2], F32, tag="gW2")
            tH = spool.tile([128, NB2], F32, tag="gH")
            dve.tensor_scalar(tA[:], idf_t[:], 2.0, None, ALU.add)
            dve.reciprocal(tR[:], tA[:])
            dve.tensor_tensor(tW[:], idf_t[:], tR[:], ALU.mult)
            dve.tensor_tensor(tW2[:], tW[:], tW[:], ALU.mult)
            dve.memset(tH[:], 1.0 / 9.0)
            for cc in (1.0 / 7.0, 1.0 / 5.0, 1.0 / 3.0, 1.0):
                dve.tensor_tensor(tH[:], tH[:], tW2[:], ALU.mult)
                dve.tensor_scalar(tH[:], tH[:], cc, None, ALU.add)
            dve.tensor_tensor(tH[:], tH[:], tW[:], ALU.mult)
            dve.tensor_scalar(tH[:], tH[:], 2.0, None, ALU.mult)
            dve.tensor_scalar(
                tH[:], tH[:], sc_b[:, 0:1], sc_b[:, 1:2], ALU.mult, ALU.add
            )
            gate_t = spool.tile([128, NB2], F32, tag="gate")
            nc.scalar.activation(gate_t[:], tH[:], AF.Tanh, scale=0.5)
            dve.tensor_scalar(gate_t[:], gate_t[:], 0.5, 0.5, ALU.mult, ALU.add)
            gm_t = spool.tile([128, NB2], F32, tag="gm")
            dve.tensor_tensor(gm_t[:], gate_t[:], msk_t[:], ALU.mult)

            # q path (all b at once)
            qeT = spool.tile([128, 6, BPC], BF16, tag="qeT")
            with nc.allow_non_contiguous_dma("qeT transposed load"):
                for c in range(6):
                    sdma.dma_start(
                        qeT[:, c, :],
                        q8_d[:, c * 128 : (c + 1) * 128].rearrange("j p -> p j"),
                    )
            qgT = spool.tile([128, H * BPC], BF16, tag="qgT")
            qgF = spool.tile([128, H * BPC], F32, tag="qgF")
            for h in range(H):
                psq = pset.tile([128, BPC], F32, tag="ps_q")
                for c in range(6):
                    nc.tensor.matmul(
                        psq[:],
                        wq_t[c][:, h * 128 : (h + 1) * 128],
                        qeT[:, c, :],
                        start=(c == 0),
                        stop=(c == 5 and not have["bq"]),
                    )
                if have["bq"]:
                    nc.tensor.matmul(
                        psq[:],
                        brow_t[1][:, h * 128 : (h + 1) * 128],
                        ones1_128b[:, 0:BPC],
                        start=False,
                        stop=True,
                    )
                nc.scalar.activation(
                    qgT[:, h * BPC : (h + 1) * BPC], psq[:], AF.Gelu
                )
                dve.tensor_copy(
                    qgF[:, h * BPC : (h + 1) * BPC],
                    qgT[:, h * BPC : (h + 1) * BPC],
                )
            # qb_all[b, h, dup, :] = q_bh @ Wa (+ s1_b); duplicated x2 cols
            qb_all = spool.tile([BPC, H, 2, 2 * D], BF16, tag="qb_all")
            for h in range(H):
                psqb = pset.tile([BPC, 2 * D], F32, tag="ps_qb")
                nc.tensor.matmul(
                    psqb[:],
                    qgT[:, h * BPC : (h + 1) * BPC],
                    wa_t[:],
                    start=True,
                    stop=not have["s1b"],
                )
                if have["s1b"]:
                    nc.tensor.matmul(
                        psqb[:],
                        ones1_128b[:, 0:BPC],
                        brow_t[2][:, 0 : 2 * D],
                        start=False,
                        stop=True,
                    )
                dve.tensor_copy(qb_all[:, h, 0, :], psqb[:])
                dve.tensor_copy(qb_all[:, h, 1, :], psqb[:])

        gatedc = spool.tile([128, BPC * 16], F32, tag="gatedc")

        # ---------- main pipeline ----------
        with (
            tc.tile_pool(name="work", bufs=2) as wpool,
            tc.tile_pool(name="stats", bufs=2) as stp,
            tc.tile_pool(name="psum_main", bufs=2, space="PSUM") as pp,
            tc.tile_pool(name="psum_m3", bufs=3, space="PSUM") as pp3,
            tc.tile_pool(name="psum_t1", bufs=1, space="PSUM") as pp1,
        ):
            for b in range(BPC):
                # qterm rows for this b gathered onto partition 0: [1,(h,2*256)]
                qbrow = wpool.tile([1, H, 2 * 2 * D], BF16, tag="qbrow")
                sdma.dma_start(
                    qbrow[:],
                    qb_all[b : b + 1, :, :, :].rearrange("o h a n -> o h (a n)"),
                )
                # ---- stage 1: node = LN(ent + role_emb[roles]) ----
                rolesf = wpool.tile([1, E], BF16, tag="rolesf")
                sdma.dma_start(rolesf[:], roles_d[b, :][None, :])
                statsX = stp.tile([128, 4, 6], F32, tag="statsX")
                xs = []
                for t in range(2):
                    ent_t = wpool.tile([128, EMB], BF16, tag="ent")
                    sdma.dma_start(ent_t[:], ent_d[b, t * 128 : (t + 1) * 128, :])
                    ps6 = pp1.tile([6, 128], F32, tag="pt")
                    nc.tensor.matmul(
                        ps6[:],
                        ones1_6[:],
                        rolesf[:, t * 128 : (t + 1) * 128],
                        start=True,
                        stop=True,
                    )
                    oh = wpool.tile([6, 128], BF16, tag="oh")
                    dve.tensor_tensor(oh[:], ps6[:], iota6[:], ALU.is_equal)
                    psx = pp.tile([128, EMB], F32, tag="px")
                    for c0 in range(2):
                        sl = slice(c0 * 512, 512 + c0 * 256)
                        nc.tensor.matmul(
                            psx[:, sl],
                            ident[:],
                            ent_t[:, sl],
                            start=True,
                            stop=False,
                        )
                        nc.tensor.matmul(
                            psx[:, sl], oh[:], role_t[:, sl], start=False, stop=True
                        )
                    for g in range(2):
                        dve.bn_stats(
                            statsX[:, 2 * t + g, :],
                            psx[:, g * 384 : (g + 1) * 384],
                        )
                    xs.append(psx)
                m4, v4 = pool_stats(stp, statsX[:], 4, 384, "lnx")
                mX, vX = pool_pairs(stp, m4, v4, 2, "lnx")
                dve.tensor_scalar(vX[:], vX[:], EPS, None, ALU.add)
                rX = newton_rsqrt(stp, vX, 2, "lnx")
                nmrX = stp.tile([128, 2], F32, tag="nmrX")
                dve.scalar_tensor_tensor(
                    nmrX[:], mX[:], -1.0, rX[:], ALU.mult, ALU.mult
                )
                nodeT = wpool.tile([128, 2, 6, 128], BF16, tag="nodeT")
                for t in range(2):
                    yn = wpool.tile([128, EMB], BF16, tag="yn")
                    nc.scalar.activation(
                        yn[:],
                        xs[t][:],
                        AF.Identity,
                        bias=nmrX[:, t : t + 1],
                        scale=rX[:, t : t + 1],
                    )
                    sdma.dma_start_transpose(nodeT[:, t, :, :], yn[:])

                # ---- stage 2: kT[h] = gelu(wk'.T @ nodeT) ----
                kT = []
                for hp in range(4):
                    psk = pp3.tile([128, 512], F32, tag="pm")
                    for hh in range(2):
                        h = 2 * hp + hh
                        for c in range(6):
                            nc.tensor.matmul(
                                psk[:, hh * 256 : (hh + 1) * 256],
                                wk_t[c][:, h * 128 : (h + 1) * 128],
                                nodeT[:, :, c, :],
                                start=(c == 0),
                                stop=(c == 5 and not have["bk"]),
                            )
                        if have["bk"]:
                            nc.tensor.matmul(
                                psk[:, hh * 256 : (hh + 1) * 256],
                                brow_t[0][:, h * 128 : (h + 1) * 128],
                                ones1_256b[:],
                                start=False,
                                stop=True,
                            )
                    kt = wpool.tile([128, 512], BF16, tag=f"kT{hp}")
                    nc.scalar.activation(kt[:], psk[:], AF.Gelu)
                    kT.append(kt)

                # ---- stage 3: interaction chunks (transposed) ----
                int1 = wpool.tile([128, H * E], BF16, tag="int1")
                int2 = wpool.tile([128, H * E], BF16, tag="int2")
                for h in range(H):
                    ksl = kT[h // 2][:, (h % 2) * 256 : (h % 2) * 256 + 256]
                    qcol = qgF[:, h * BPC + b : h * BPC + b + 1]
                    dve.tensor_scalar(
                        int1[:, h * 256 : (h + 1) * 256], ksl, qcol, None,
                        ALU.subtract,
                    )
                    iv = int1[:, h * 256 : (h + 1) * 256].bitcast(mybir.dt.uint16)
                    dve.tensor_scalar(iv, iv, 0x7FFF, None, ALU.bitwise_and)
                    dve.tensor_scalar(
                        int2[:, h * 256 : (h + 1) * 256],
                        ksl,
                        qcol,
                        None,
                        ALU.mult,
                    )

                # ---- stage 4: s1 row-major + gelu + stats ----
                statsY = stp.tile([128, 16, 6], F32, tag="statsY")
                ybuf = []
                for sp in range(8):
                    h = sp
                    ps1 = pp3.tile([128, 512], F32, tag="pm")
                    for half in range(2):
                        s_ = 2 * sp + half
                        base_col = (h % 2) * 256 + half * 128
                        lhs = [
                            kT[h // 2][:, base_col : base_col + 128],
                            int1[:, s_ * 128 : (s_ + 1) * 128],
                            int2[:, s_ * 128 : (s_ + 1) * 128],
                        ]
                        for kc in range(3):
                            nc.tensor.matmul(
                                ps1[:, half * 256 : (half + 1) * 256],
                                lhs[kc],
                                wbcd[kc][:],
                                start=(kc == 0),
                                stop=False,
                            )
                        nc.tensor.matmul(
                            ps1[:, half * 256 : (half + 1) * 256],
                            ones1_128b[:, 0:128],
                            qbrow[0:1, h, half * 256 : half * 256 + 256],
                            start=False,
                            stop=True,
                        )
                    yt = wpool.tile([128, 512], BF16, tag=f"y{sp}")
                    nc.scalar.activation(yt[:], ps1[:], AF.Gelu)
                    for g in range(2):
                        dve.bn_stats(
                            statsY[:, 2 * sp + g, :],
                            yt[:, g * 256 : (g + 1) * 256],
                        )
                    ybuf.append(yt)
                mY, vY = pool_stats(stp, statsY[:], 16, 256, "ln1")
                dve.tensor_scalar(vY[:], vY[:], EPS, None, ALU.add)
                rY = newton_rsqrt(stp, vY, 16, "ln1")
                nmrY = stp.tile([128, 16], F32, tag="nmrY")
                dve.scalar_tensor_tensor(
                    nmrY[:], mY[:], -1.0, rY[:], ALU.mult, ALU.mult
                )

                # ---- stage 5: normalize, transpose, s2, gelu, stats ----
                ynT = wpool.tile([128, 32, 128], BF16, tag="ynT")
                ynbig = wpool.tile([128, 16 * 256], BF16, tag="ynbig")
                for s_ in range(16):
                    dve.tensor_scalar(
                        ynbig[:, s_ * 256 : (s_ + 1) * 256],
                        ybuf[s_ // 2][:, (s_ % 2) * 256 : (s_ % 2) * 256 + 256],
                        rY[:, s_ : s_ + 1],
                        nmrY[:, s_ : s_ + 1],
                        ALU.mult,
                        ALU.add,
                    )
                sdma.dma_start_transpose(ynT[:], ynbig[:])
                statsZ = stp.tile([128, 16, 6], F32, tag="statsZ")
                zbuf = []
                for zt in range(4):
                    ps2 = pp3.tile([128, 512], F32, tag="pm")
                    for j in range(4):
                        s_ = zt * 4 + j
                        for kc in range(2):
                            nc.tensor.matmul(
                                ps2[:, j * 128 : (j + 1) * 128],
                                ynT[:, s_ * 2 + kc, :],
                                w2_t[kc][:],
                                start=(kc == 0),
                                stop=(kc == 1 and not have["s2b"]),
                            )
                        if have["s2b"]:
                            nc.tensor.matmul(
                                ps2[:, j * 128 : (j + 1) * 128],
                                ones1_128b[:, 0:128],
                                brow_t[3][:, 0:128],
                                start=False,
                                stop=True,
                            )
                    ztile = wpool.tile([128, 512], BF16, tag=f"z{zt}")
                    nc.scalar.activation(ztile[:], ps2[:], AF.Gelu)
                    for g in range(4):
                        dve.bn_stats(
                            statsZ[:, zt * 4 + g, :],
                            ztile[:, g * 128 : (g + 1) * 128],
                        )
                    zbuf.append(ztile)
                mZ, vZ = pool_stats(stp, statsZ[:], 16, D, "ln2")
                dve.tensor_scalar(vZ[:], vZ[:], EPS, None, ALU.add)
                rZ = newton_rsqrt(stp, vZ, 16, "ln2")

                # ---- stage 6: s3 dot + base + gate ----
                tcol = stp.tile([128, 16], F32, tag="tcol")
                for zt in range(4):
                    wsc = wpool.tile([128, 512], BF16, tag="wsc")
                    dve.tensor_tensor(wsc[:], zbuf[zt][:], s3w_b4[:], ALU.mult)
                    dve.reduce_sum(
                        tcol[:, zt * 4 : (zt + 1) * 4],
                        wsc[:].rearrange("p (g n) -> p g n", g=4),
                        axis=AX.X,
                    )
                base = stp.tile([128, 16], F32, tag="base")
                u = stp.tile([128, 16], F32, tag="ubase")
                dve.tensor_scalar(u[:], mZ[:], w3bar[:, 0:1], None, ALU.mult)
                dve.tensor_tensor(base[:], tcol[:], u[:], ALU.subtract)
                dve.tensor_tensor(base[:], base[:], rZ[:], ALU.mult)
                if have["s3c"]:
                    dve.tensor_scalar(base[:], base[:], sc_b[:, 2:3], None, ALU.add)
                bv = base[:].rearrange("p (h e) -> p h e", e=2)
                gv = gatedc[:, b * 16 : (b + 1) * 16].rearrange(
                    "p (h e) -> p h e", e=2
                )
                for eh in range(2):
                    dve.tensor_scalar(
                        gv[:, :, eh],
                        bv[:, :, eh],
                        gm_t[:, b * 2 + eh : b * 2 + eh + 1],
                        None,
                        ALU.mult,
                    )

            # ---- final reduction ----
            psf = pp1.tile([1, BPC * 16], F32, tag="pt")
            nc.tensor.matmul(psf[:], ones128_f[:], gatedc[:], start=True, stop=True)
            hrow = spool.tile([1, BPC * 16], F32, tag="hrow")
            dve.tensor_copy(hrow[:], psf[:])
            srow = spool.tile([1, BPC], F32, tag="srow")
            dve.reduce_sum(
                srow[:], hrow[:].rearrange("p (b g) -> p b g", g=16), axis=AX.X
            )
            orow = spool.tile([1, BPC], F32, tag="orow")
            nc.scalar.activation(orow[:], srow[:], AF.Tanh, scale=1.0 / 16.0)
            dve.tensor_scalar(orow[:], orow[:], 0.5, 0.5, ALU.mult, ALU.add)
            sdma.dma_start(out_d[:], orow[:])

    nc.compile()
    return nc


def make_in_maps(inputs, fp):
    ent = _f32(inputs["ent_embs"]).reshape(NCORES, BPC, E, EMB)
    q8 = _f32(inputs["q_emb"]).reshape(NCORES, BPC, EMB)
    roles = np.asarray(inputs["roles"]).reshape(NCORES, BPC, E)
    idfs = _f32(inputs["idfs"]).reshape(NCORES, BPC, E)
    mask = np.asarray(inputs["mask"]).reshape(NCORES, BPC, E)
    scal = np.zeros(8, np.float32)
    scal[0] = fp["idf_w"]
    scal[1] = fp["idf_b"]
    scal[2] = fp["s3c"]
    brows = np.zeros((4, H * D), np.float32)
    brows[0] = fp["bk"]
    brows[1] = fp["bq"]
    brows[2, 0 : 2 * D] = fp["s1b"]
    brows[3, 0 : 4 * D] = np.tile(fp["s2b"], 4)

    shared = {
        "wk": fp["wk"],
        "wq": fp["wq"],
        "wa": fp["wa"],
        "wb": fp["wb"],
        "wc": fp["wc"],
        "wd": fp["wd"],
        "w2": fp["w2"],
        "s3w4": fp["s3w4"].reshape(1, -1),
        "role_emb": fp["role_emb"].astype(BF),
        "brows": brows.astype(BF),
        "scal": scal.reshape(1, 8),
        "iota6": np.repeat(np.arange(6, dtype=np.float32), 128).reshape(6, 128).astype(BF),
        "ident": np.eye(128, dtype=np.float32).astype(BF),
    }
    maps = []
    for c in range(NCORES):
        m = dict(shared)
        m["ent"] = ent[c].astype(BF)
        m["q8"] = q8[c].astype(BF)
        m["rolesf"] = roles[c].astype(BF)
        m["idfs"] = idfs[c]
        m["maskf"] = mask[c].astype(np.float32)
        maps.append(m)
    return maps


def _have_flags(fp):
    return {
        "bk": bool(np.any(fp["bk"])),
        "bq": bool(np.any(fp["bq"])),
        "s1b": bool(np.any(fp["s1b"])),
        "s2b": bool(np.any(fp["s2b"])),
        "s3c": bool(fp["s3c"] != 0.0),
    }


_CACHE = {}


def kernel(**inputs):
    fp = fold_params(inputs)
    have = _have_flags(fp)
    key = tuple(sorted(have.items()))
    if key not in _CACHE:
        _CACHE[key] = build_program(have)
    nc = _CACHE[key]
    in_maps = make_in_maps(inputs, fp)
    res = run_bass_kernel_spmd(nc, in_maps, list(range(NCORES)))
    out = np.concatenate([res.results[c]["out"] for c in range(NCORES)])
    return out.astype(np.float32)
